# revision 1
# baseline (speedup 1.0000x reference)
"""Trainium2 Bass kernel for causal multi-head attention.

Problem: x[64,256,512] f32, Wq/Wk/Wv[8,512,64], Wo[512,512]
  q,k,v = einsum('btc,hcd->bhtd'); scores = q k^T / sqrt(512) (causal);
  out = softmax(scores) v; y = concat-heads(out) @ Wo.

Strategy: data-parallel over batch across 8 NeuronCores (8 batches/core,
no collectives). Per core, for each batch b:
  - load host-pretransposed xT [512c, 256t]
  - QT/KT = W^T x^T via head-pair-packed matmuls -> [d, t] layout
  - V' = x Wv -> [s, h, d] layout with a ones column appended per head
    (PV matmul then yields the softmax denominator for free)
  - S^T[s,t] = KT^T QT per head; additive causal mask on the two diagonal
    128x128 blocks (DVE); P = exp(scale * S~) on ACT (scale=1/sqrt(512))
  - PV: out'[t, 65] = P^T.T V' accumulated over s-chunks; col 64 = denom
  - normalize rows by 1/denom during the PSUM->SBUF copy (ACT, per-
    partition scale) -> out[t, hd]
  - PE-transpose out -> outT [hd, t]; y = outT^T Wo; DMA out.

The fully-masked (t-block0, s-block1) region is never computed.
Precision is configurable per stage (fp32 native matmul = 4 cyc/row,
fp16 = 1 cyc/row).
"""
import numpy as np

import concourse.bass as bass
import concourse.tile as tile
import concourse.mybir as mybir
from concourse import bacc
from concourse.bass_utils import run_bass_kernel_spmd

F32 = mybir.dt.float32
F16 = mybir.dt.float16
BF16 = mybir.dt.bfloat16

N_CORES = 8
B, T, C = 64, 256, 512
H, DK = 8, 64
B_LOC = B // N_CORES        # 8 batches per core
N_HP = H // 2               # head pairs (2x64 packed on partitions)
N_CC = C // 128             # contraction chunks
SCALE = 1.0 / np.sqrt(np.float32(C))
NEG = -1.0e30

_DT = {"fp32": F32, "fp16": F16, "bf16": BF16}


def build_nc(cfg, repeat=0, tune=None):
    """repeat=0: straight-line kernel. repeat=R>0: wrap the whole per-batch
    pipeline in a hardware For_i loop executed R times (for timing)."""
    tu = {"xT": 2, "qk": 2, "vp": 2, "pp": 4, "op": 2, "rp": 8, "yp": 2,
          "ps_proj": 2, "ps_s": 3, "ps_pv": 2, "ps_tr": 1}
    tu.update(tune or {})
    mask_sbuf = bool(cfg.get("mask_sbuf", False))
    proj_dt = _DT[cfg["proj"]]     # x / Wq / Wk / Wv operand dtype
    sc_dt = _DT[cfg["scores"]]     # QT / KT operand dtype
    pv_dt = _DT[cfg["pv"]]         # P / V' operand dtype
    op_dt = _DT[cfg["outproj"]]    # outT / Wo operand dtype

    nc = bacc.Bacc("TRN2", target_bir_lowering=False, debug=False)

    xT_d = nc.dram_tensor("xT", [B_LOC, C, T], proj_dt, kind="ExternalInput").ap()
    wq_d = nc.dram_tensor("wq", [128, N_HP * N_CC * 128], proj_dt, kind="ExternalInput").ap()
    wk_d = nc.dram_tensor("wk", [128, N_HP * N_CC * 128], proj_dt, kind="ExternalInput").ap()
    wv_d = nc.dram_tensor("wv", [128, N_CC * C], proj_dt, kind="ExternalInput").ap()
    wo_d = nc.dram_tensor("wo", [128, N_CC * C], op_dt, kind="ExternalInput").ap()
    mask_d = nc.dram_tensor("mask", [128, 128], F32, kind="ExternalInput").ap()
    ident_d = nc.dram_tensor("ident", [128, 128], F32, kind="ExternalInput").ap()
    y_d = nc.dram_tensor("y", [B_LOC, T, C], F32, kind="ExternalOutput").ap()

    with tile.TileContext(nc) as tc:
        import contextlib
        ctx = contextlib.ExitStack()
        with ctx:
            const = ctx.enter_context(tc.tile_pool(name="const", bufs=1))
            xT_p = ctx.enter_context(tc.tile_pool(name="xT", bufs=tu["xT"]))
            qk_p = ctx.enter_context(tc.tile_pool(name="qk", bufs=tu["qk"]))
            vp_p = ctx.enter_context(tc.tile_pool(name="vp", bufs=tu["vp"]))
            p_p = ctx.enter_context(tc.tile_pool(name="pp", bufs=tu["pp"]))
            o_p = ctx.enter_context(tc.tile_pool(name="op", bufs=tu["op"]))
            r_p = ctx.enter_context(tc.tile_pool(name="rp", bufs=tu["rp"]))
            y_p = ctx.enter_context(tc.tile_pool(name="yp", bufs=tu["yp"]))
            ps_proj = ctx.enter_context(tc.tile_pool(name="ps_proj", bufs=tu["ps_proj"], space="PSUM"))
            ps_s = ctx.enter_context(tc.tile_pool(name="ps_s", bufs=tu["ps_s"], space="PSUM"))
            ps_pv = ctx.enter_context(tc.tile_pool(name="ps_pv", bufs=tu["ps_pv"], space="PSUM"))
            ps_tr = ctx.enter_context(tc.tile_pool(name="ps_tr", bufs=tu["ps_tr"], space="PSUM"))

            # persistent constants / weights
            mask = const.tile([128, 128], F32)
            nc.sync.dma_start(mask[:], mask_d[:])
            ident = const.tile([128, 128], F32)
            nc.sync.dma_start(ident[:], ident_d[:])
            wq = const.tile([128, N_HP, N_CC, 128], proj_dt)
            nc.sync.dma_start(wq[:], wq_d.rearrange("p (a b c) -> p a b c", a=N_HP, b=N_CC))
            wk = const.tile([128, N_HP, N_CC, 128], proj_dt)
            nc.sync.dma_start(wk[:], wk_d.rearrange("p (a b c) -> p a b c", a=N_HP, b=N_CC))
            wv = const.tile([128, N_CC, C], proj_dt)
            nc.sync.dma_start(wv[:], wv_d.rearrange("p (a b) -> p a b", a=N_CC))
            wo = const.tile([128, N_CC, C], op_dt)
            nc.sync.dma_start(wo[:], wo_d.rearrange("p (a b) -> p a b", a=N_CC))

            def batch_pipeline():
              for b in range(B_LOC):
                  # ---- load xT for this batch: [128, cc, 256] ----
                  xT = xT_p.tile([128, N_CC, T], proj_dt, tag="xT")
                  nc.sync.dma_start(
                      xT[:], xT_d[b].rearrange("(a p) t -> p a t", p=128))

                  # ---- Q/K projections -> QT/KT [128(2x64d), hp, 256t] ----
                  qt = qk_p.tile([128, N_HP, T], sc_dt, tag="qt")
                  kt = qk_p.tile([128, N_HP, T], sc_dt, tag="kt")
                  for (w, dst) in ((wq, qt), (wk, kt)):
                      for hp in range(N_HP):
                          ps = ps_proj.tile([128, 512], F32, tag="ps_proj")
                          for cc in range(N_CC):
                              nc.tensor.matmul(
                                  ps[:, 0:T], w[:, hp, cc, :], xT[:, cc, :],
                                  start=(cc == 0), stop=(cc == N_CC - 1))
                          nc.scalar.copy(dst[:, hp, :], ps[:, 0:T])

                  # ---- V projection -> V' [128s, schunk, h, 65] ----
                  vv = vp_p.tile([128, 2, H, 65], pv_dt, tag="vv")
                  for sc in range(2):
                      ps = ps_proj.tile([128, 512], F32, tag="ps_proj")
                      for cc in range(N_CC):
                          nc.tensor.matmul(
                              ps[:], xT[:, cc, bass.ts(sc, 128)], wv[:, cc, :],
                              start=(cc == 0), stop=(cc == N_CC - 1))
                      nc.scalar.copy(
                          vv[:, sc, :, 0:DK],
                          ps[:].rearrange("p (h d) -> p h d", h=H))
                      nc.vector.memset(vv[:, sc, :, DK:65], 1.0)

                  # ---- attention per head ----
                  out = o_p.tile([128, 2, C], F32, tag="out")
                  for h in range(H):
                      hp, lo = h // 2, (h % 2) * DK
                      qs = qt[lo:lo + DK, hp, :]
                      ks = kt[lo:lo + DK, hp, :]

                      # S^T tiles: s0 x all t; s1 x t1 only
                      s0 = ps_s.tile([128, 256], F32, tag="s")
                      nc.tensor.matmul(s0[:], ks[:, 0:128], qs[:],
                                       start=True, stop=True)
                      s1f = ps_s.tile([128, 256], F32, tag="s", name="s1f")
                      s1 = s1f[:, 0:128]
                      nc.tensor.matmul(s1[:], ks[:, 128:256], qs[:, 128:256],
                                       start=True, stop=True)
                      if not mask_sbuf:
                          # causal mask: additive -1e30 on PSUM before exp
                          nc.vector.tensor_add(s0[:, 0:128], s0[:, 0:128],
                                               mask[:])
                          nc.vector.tensor_add(s1[:], s1[:], mask[:])
                      # P = exp(scale * S~)
                      p0 = p_p.tile([128, 256], pv_dt, tag="p0")
                      nc.scalar.activation(p0[:], s0[:],
                                           mybir.ActivationFunctionType.Exp,
                                           bias=0.0, scale=float(SCALE))
                      p1 = p_p.tile([128, 128], pv_dt, tag="p1")
                      nc.scalar.activation(p1[:], s1[:],
                                           mybir.ActivationFunctionType.Exp,
                                           bias=0.0, scale=float(SCALE))
                      if mask_sbuf:
                          # causal mask: multiplicative binary on SBUF after
                          # exp (exp of unmasked scores is bounded; exact)
                          nc.vector.tensor_mul(p0[:, 0:128], p0[:, 0:128],
                                               mask[:])
                          nc.vector.tensor_mul(p1[:], p1[:], mask[:])

                      # PV: out'[t,65], col 64 = denominator
                      pv0 = ps_pv.tile([128, 65], F32, tag="pv")
                      nc.tensor.matmul(pv0[:], p0[:, 0:128], vv[:, 0, h, :],
                                       start=True, stop=True)
                      pv1 = ps_pv.tile([128, 65], F32, tag="pv")
                      nc.tensor.matmul(pv1[:], p0[:, 128:256], vv[:, 0, h, :],
                                       start=True, stop=False)
                      nc.tensor.matmul(pv1[:], p1[:], vv[:, 1, h, :],
                                       start=False, stop=True)
                      for tb, pv in ((0, pv0), (1, pv1)):
                          rec = r_p.tile([128, 1], F32, tag="rec")
                          nc.vector.reciprocal(rec[:], pv[:, DK:65])
                          nc.scalar.activation(
                              out[:, tb, h * DK:(h + 1) * DK], pv[:, 0:DK],
                              mybir.ActivationFunctionType.Copy,
                              bias=0.0, scale=rec[:])

                  # ---- transpose out [t,hd] -> outT [hd, tb*128] ----
                  outT = o_p.tile([128, N_CC, T], op_dt, tag="outT")
                  for cco in range(N_CC):
                      for tb in range(2):
                          tp = ps_tr.tile([128, 128], F32, tag="tp")
                          nc.tensor.transpose(
                              tp[:], out[:, tb, bass.ts(cco, 128)], ident[:])
                          nc.scalar.copy(outT[:, cco, bass.ts(tb, 128)], tp[:])

                  # ---- output projection ----
                  for tb in range(2):
                      ps = ps_proj.tile([128, 512], F32, tag="ps_proj")
                      for cc in range(N_CC):
                          nc.tensor.matmul(
                              ps[:], outT[:, cc, bass.ts(tb, 128)], wo[:, cc, :],
                              start=(cc == 0), stop=(cc == N_CC - 1))
                      yt = y_p.tile([128, C], F32, tag="yt")
                      nc.scalar.copy(yt[:], ps[:])
                      nc.sync.dma_start(y_d[b, bass.ts(tb, 128), :], yt[:])


            if repeat:
                with tc.For_i(0, repeat, 1):
                    batch_pipeline()
            else:
                batch_pipeline()

    nc.compile()
    return nc


def _prep_inputs(x, Wq, Wk, Wv, Wo, cfg):
    """Host-side reshapes/casts. Returns per-core input maps."""
    proj_np = np.float16 if cfg["proj"] == "fp16" else np.float32
    op_np = np.float16 if cfg["outproj"] == "fp16" else np.float32
    if cfg["proj"] == "bf16" or cfg["outproj"] == "bf16":
        import ml_dtypes
        if cfg["proj"] == "bf16":
            proj_np = ml_dtypes.bfloat16
        if cfg["outproj"] == "bf16":
            op_np = ml_dtypes.bfloat16

    # weights: head-pair stationary blocks [hp, cc, 128c, 128d2] -> [128, hp*cc*128]
    def pack_qk(w):
        w2 = np.ascontiguousarray(w.transpose(1, 0, 2)).reshape(C, C)  # [c, h*64]
        w4 = w2.reshape(N_CC, 128, N_HP, 128).transpose(1, 2, 0, 3)   # [128c, hp, cc, 128]
        return np.ascontiguousarray(w4).reshape(128, -1).astype(proj_np)

    wq_h = pack_qk(Wq)
    wk_h = pack_qk(Wk)
    wv2 = np.ascontiguousarray(Wv.transpose(1, 0, 2)).reshape(C, C)    # [c, hd]
    wv_h = np.ascontiguousarray(
        wv2.reshape(N_CC, 128, C).transpose(1, 0, 2)).reshape(128, -1).astype(proj_np)
    wo_h = np.ascontiguousarray(
        Wo.reshape(N_CC, 128, C).transpose(1, 0, 2)).reshape(128, -1).astype(op_np)

    ii, jj = np.indices((128, 128))
    if cfg.get("mask_sbuf", False):
        mask_h = (jj >= ii).astype(np.float32)
    else:
        mask_h = np.where(jj >= ii, 0.0, NEG).astype(np.float32)
    ident_h = np.eye(128, dtype=np.float32)

    in_maps = []
    for core in range(N_CORES):
        xs = x[core * B_LOC:(core + 1) * B_LOC]              # [8, 256, 512]
        xT = np.ascontiguousarray(xs.transpose(0, 2, 1)).astype(proj_np)
        in_maps.append({
            "xT": xT, "wq": wq_h, "wk": wk_h, "wv": wv_h, "wo": wo_h,
            "mask": mask_h, "ident": ident_h,
        })
    return in_maps


DEFAULT_CFG = {"proj": "fp32", "scores": "fp32", "pv": "fp32", "outproj": "fp32"}

_NC_CACHE = {}


def run(x, Wq, Wk, Wv, Wo, cfg=None, trace=False):
    cfg = cfg or DEFAULT_CFG
    key = tuple(sorted(cfg.items()))
    if key not in _NC_CACHE:
        _NC_CACHE[key] = build_nc(cfg)
    nc = _NC_CACHE[key]
    in_maps = _prep_inputs(np.asarray(x), np.asarray(Wq), np.asarray(Wk),
                           np.asarray(Wv), np.asarray(Wo), cfg)
    res = run_bass_kernel_spmd(nc, in_maps, core_ids=list(range(N_CORES)),
                               trace=trace)
    y = np.concatenate([r["y"] for r in res.results], axis=0)
    return y, res


def kernel(x, Wq, Wk, Wv, Wo):
    y, _ = run(x, Wq, Wk, Wv, Wo)
    return y.astype(np.float32)


if __name__ == "__main__":
    import time
    t0 = time.time()
    nc = build_nc(DEFAULT_CFG)
    print(f"build+compile: {time.time()-t0:.1f}s")



# revision 2
# speedup vs baseline: 2.3813x; 2.3813x over previous
"""Trainium2 Bass kernel for causal multi-head attention.

Problem: x[64,256,512] f32, Wq/Wk/Wv[8,512,64], Wo[512,512]
  q,k,v = einsum('btc,hcd->bhtd'); scores = q k^T / sqrt(512) (causal);
  out = softmax(scores) v; y = concat-heads(out) @ Wo.

Strategy: data-parallel over batch across 8 NeuronCores (8 batches/core,
no collectives). Per core, for each batch b:
  - load host-pretransposed xT [512c, 256t]
  - QT/KT = W^T x^T via head-pair-packed matmuls -> [d, t] layout
  - V' = x Wv -> [s, h, d] layout with a ones column appended per head
    (PV matmul then yields the softmax denominator for free)
  - S^T[s,t] = KT^T QT per head; additive causal mask on the two diagonal
    128x128 blocks (DVE); P = exp(scale * S~) on ACT (scale=1/sqrt(512))
  - PV: out'[t, 65] = P^T.T V' accumulated over s-chunks; col 64 = denom
  - normalize rows by 1/denom during the PSUM->SBUF copy (ACT, per-
    partition scale) -> out[t, hd]
  - PE-transpose out -> outT [hd, t]; y = outT^T Wo; DMA out.

The fully-masked (t-block0, s-block1) region is never computed.
Precision is configurable per stage (fp32 native matmul = 4 cyc/row,
fp16 = 1 cyc/row).
"""
import numpy as np

import concourse.bass as bass
import concourse.tile as tile
import concourse.mybir as mybir
from concourse import bacc
from concourse.bass_utils import run_bass_kernel_spmd

F32 = mybir.dt.float32
F16 = mybir.dt.float16
BF16 = mybir.dt.bfloat16

N_CORES = 8
B, T, C = 64, 256, 512
H, DK = 8, 64
B_LOC = B // N_CORES        # 8 batches per core
N_HP = H // 2               # head pairs (2x64 packed on partitions)
N_CC = C // 128             # contraction chunks
SCALE = 1.0 / np.sqrt(np.float32(C))
NEG = -1.0e30

_DT = {"fp32": F32, "fp16": F16, "bf16": BF16}


def build_nc(cfg, repeat=0, tune=None):
    """repeat=0: straight-line kernel. repeat=R>0: wrap the whole per-batch
    pipeline in a hardware For_i loop executed R times (for timing)."""
    tu = {"xT": 2, "qk": 2, "vp": 2, "pp": 4, "op": 2, "rp": 8, "yp": 2,
          "ps_proj": 2, "ps_s": 3, "ps_pv": 2, "ps_tr": 1}
    tu.update(tune or {})
    mask_sbuf = bool(cfg.get("mask_sbuf", False))
    proj_dt = _DT[cfg["proj"]]     # x / Wq / Wk / Wv operand dtype
    sc_dt = _DT[cfg["scores"]]     # QT / KT operand dtype
    pv_dt = _DT[cfg["pv"]]         # P / V' operand dtype
    op_dt = _DT[cfg["outproj"]]    # outT / Wo operand dtype

    nc = bacc.Bacc("TRN2", target_bir_lowering=False, debug=False)

    xT_d = nc.dram_tensor("xT", [B_LOC, C, T], proj_dt, kind="ExternalInput").ap()
    wq_d = nc.dram_tensor("wq", [128, N_HP * N_CC * 128], proj_dt, kind="ExternalInput").ap()
    wk_d = nc.dram_tensor("wk", [128, N_HP * N_CC * 128], proj_dt, kind="ExternalInput").ap()
    wv_d = nc.dram_tensor("wv", [128, N_CC * C], proj_dt, kind="ExternalInput").ap()
    wo_d = nc.dram_tensor("wo", [128, N_CC * C], op_dt, kind="ExternalInput").ap()
    mask_d = nc.dram_tensor("mask", [128, 128], F32, kind="ExternalInput").ap()
    ident_d = nc.dram_tensor("ident", [128, 128], F32, kind="ExternalInput").ap()
    y_d = nc.dram_tensor("y", [B_LOC, T, C], F32, kind="ExternalOutput").ap()

    with tile.TileContext(nc) as tc:
        import contextlib
        ctx = contextlib.ExitStack()
        with ctx:
            const = ctx.enter_context(tc.tile_pool(name="const", bufs=1))
            xT_p = ctx.enter_context(tc.tile_pool(name="xT", bufs=tu["xT"]))
            qk_p = ctx.enter_context(tc.tile_pool(name="qk", bufs=tu["qk"]))
            vp_p = ctx.enter_context(tc.tile_pool(name="vp", bufs=tu["vp"]))
            p_p = ctx.enter_context(tc.tile_pool(name="pp", bufs=tu["pp"]))
            o_p = ctx.enter_context(tc.tile_pool(name="op", bufs=tu["op"]))
            r_p = ctx.enter_context(tc.tile_pool(name="rp", bufs=tu["rp"]))
            y_p = ctx.enter_context(tc.tile_pool(name="yp", bufs=tu["yp"]))
            ps_proj = ctx.enter_context(tc.tile_pool(name="ps_proj", bufs=tu["ps_proj"], space="PSUM"))
            ps_s = ctx.enter_context(tc.tile_pool(name="ps_s", bufs=tu["ps_s"], space="PSUM"))
            ps_pv = ctx.enter_context(tc.tile_pool(name="ps_pv", bufs=tu["ps_pv"], space="PSUM"))
            ps_tr = ctx.enter_context(tc.tile_pool(name="ps_tr", bufs=tu["ps_tr"], space="PSUM"))

            # persistent constants / weights
            mask = const.tile([128, 128], F32)
            nc.sync.dma_start(mask[:], mask_d[:])
            ident = const.tile([128, 128], F32)
            nc.sync.dma_start(ident[:], ident_d[:])
            wq = const.tile([128, N_HP, N_CC, 128], proj_dt)
            nc.sync.dma_start(wq[:], wq_d.rearrange("p (a b c) -> p a b c", a=N_HP, b=N_CC))
            wk = const.tile([128, N_HP, N_CC, 128], proj_dt)
            nc.sync.dma_start(wk[:], wk_d.rearrange("p (a b c) -> p a b c", a=N_HP, b=N_CC))
            wv = const.tile([128, N_CC, C], proj_dt)
            nc.sync.dma_start(wv[:], wv_d.rearrange("p (a b) -> p a b", a=N_CC))
            wo = const.tile([128, N_CC, C], op_dt)
            nc.sync.dma_start(wo[:], wo_d.rearrange("p (a b) -> p a b", a=N_CC))

            def batch_pipeline():
              for b in range(B_LOC):
                  # ---- load xT for this batch: [128, cc, 256] ----
                  xT = xT_p.tile([128, N_CC, T], proj_dt, tag="xT")
                  nc.sync.dma_start(
                      xT[:], xT_d[b].rearrange("(a p) t -> p a t", p=128))

                  # ---- Q/K projections -> QT/KT [128(2x64d), hp, 256t] ----
                  qt = qk_p.tile([128, N_HP, T], sc_dt, tag="qt")
                  kt = qk_p.tile([128, N_HP, T], sc_dt, tag="kt")
                  for (w, dst) in ((wq, qt), (wk, kt)):
                      for hp in range(N_HP):
                          ps = ps_proj.tile([128, 512], F32, tag="ps_proj")
                          for cc in range(N_CC):
                              nc.tensor.matmul(
                                  ps[:, 0:T], w[:, hp, cc, :], xT[:, cc, :],
                                  start=(cc == 0), stop=(cc == N_CC - 1))
                          nc.scalar.copy(dst[:, hp, :], ps[:, 0:T])

                  # ---- V projection -> V' [128s, schunk, h, 65] ----
                  vv = vp_p.tile([128, 2, H, 65], pv_dt, tag="vv")
                  for sc in range(2):
                      ps = ps_proj.tile([128, 512], F32, tag="ps_proj")
                      for cc in range(N_CC):
                          nc.tensor.matmul(
                              ps[:], xT[:, cc, bass.ts(sc, 128)], wv[:, cc, :],
                              start=(cc == 0), stop=(cc == N_CC - 1))
                      nc.scalar.copy(
                          vv[:, sc, :, 0:DK],
                          ps[:].rearrange("p (h d) -> p h d", h=H))
                      nc.vector.memset(vv[:, sc, :, DK:65], 1.0)

                  # ---- attention per head ----
                  out = o_p.tile([128, 2, C], F32, tag="out")
                  for h in range(H):
                      hp, lo = h // 2, (h % 2) * DK
                      qs = qt[lo:lo + DK, hp, :]
                      ks = kt[lo:lo + DK, hp, :]

                      # S^T tiles: s0 x all t; s1 x t1 only
                      s0 = ps_s.tile([128, 256], F32, tag="s")
                      nc.tensor.matmul(s0[:], ks[:, 0:128], qs[:],
                                       start=True, stop=True)
                      s1f = ps_s.tile([128, 256], F32, tag="s", name="s1f")
                      s1 = s1f[:, 0:128]
                      nc.tensor.matmul(s1[:], ks[:, 128:256], qs[:, 128:256],
                                       start=True, stop=True)
                      if not mask_sbuf:
                          # causal mask: additive -1e30 on PSUM before exp
                          nc.vector.tensor_add(s0[:, 0:128], s0[:, 0:128],
                                               mask[:])
                          nc.vector.tensor_add(s1[:], s1[:], mask[:])
                      # P = exp(scale * S~)
                      p0 = p_p.tile([128, 256], pv_dt, tag="p0")
                      nc.scalar.activation(p0[:], s0[:],
                                           mybir.ActivationFunctionType.Exp,
                                           bias=0.0, scale=float(SCALE))
                      p1 = p_p.tile([128, 128], pv_dt, tag="p1")
                      nc.scalar.activation(p1[:], s1[:],
                                           mybir.ActivationFunctionType.Exp,
                                           bias=0.0, scale=float(SCALE))
                      if mask_sbuf:
                          # causal mask: multiplicative binary on SBUF after
                          # exp (exp of unmasked scores is bounded; exact)
                          nc.vector.tensor_mul(p0[:, 0:128], p0[:, 0:128],
                                               mask[:])
                          nc.vector.tensor_mul(p1[:], p1[:], mask[:])

                      # PV: out'[t,65], col 64 = denominator
                      pv0 = ps_pv.tile([128, 65], F32, tag="pv")
                      nc.tensor.matmul(pv0[:], p0[:, 0:128], vv[:, 0, h, :],
                                       start=True, stop=True)
                      pv1 = ps_pv.tile([128, 65], F32, tag="pv")
                      nc.tensor.matmul(pv1[:], p0[:, 128:256], vv[:, 0, h, :],
                                       start=True, stop=False)
                      nc.tensor.matmul(pv1[:], p1[:], vv[:, 1, h, :],
                                       start=False, stop=True)
                      for tb, pv in ((0, pv0), (1, pv1)):
                          rec = r_p.tile([128, 1], F32, tag="rec")
                          nc.vector.reciprocal(rec[:], pv[:, DK:65])
                          nc.scalar.activation(
                              out[:, tb, h * DK:(h + 1) * DK], pv[:, 0:DK],
                              mybir.ActivationFunctionType.Copy,
                              bias=0.0, scale=rec[:])

                  # ---- transpose out [t,hd] -> outT [hd, tb*128] ----
                  outT = o_p.tile([128, N_CC, T], op_dt, tag="outT")
                  for cco in range(N_CC):
                      for tb in range(2):
                          tp = ps_tr.tile([128, 128], F32, tag="tp")
                          nc.tensor.transpose(
                              tp[:], out[:, tb, bass.ts(cco, 128)], ident[:])
                          nc.scalar.copy(outT[:, cco, bass.ts(tb, 128)], tp[:])

                  # ---- output projection ----
                  for tb in range(2):
                      ps = ps_proj.tile([128, 512], F32, tag="ps_proj")
                      for cc in range(N_CC):
                          nc.tensor.matmul(
                              ps[:], outT[:, cc, bass.ts(tb, 128)], wo[:, cc, :],
                              start=(cc == 0), stop=(cc == N_CC - 1))
                      yt = y_p.tile([128, C], F32, tag="yt")
                      nc.scalar.copy(yt[:], ps[:])
                      nc.sync.dma_start(y_d[b, bass.ts(tb, 128), :], yt[:])


            if repeat:
                with tc.For_i(0, repeat, 1):
                    batch_pipeline()
            else:
                batch_pipeline()

    nc.compile()
    return nc


def _prep_inputs(x, Wq, Wk, Wv, Wo, cfg):
    """Host-side reshapes/casts. Returns per-core input maps."""
    proj_np = np.float16 if cfg["proj"] == "fp16" else np.float32
    op_np = np.float16 if cfg["outproj"] == "fp16" else np.float32
    if cfg["proj"] == "bf16" or cfg["outproj"] == "bf16":
        import ml_dtypes
        if cfg["proj"] == "bf16":
            proj_np = ml_dtypes.bfloat16
        if cfg["outproj"] == "bf16":
            op_np = ml_dtypes.bfloat16

    # weights: head-pair stationary blocks [hp, cc, 128c, 128d2] -> [128, hp*cc*128]
    def pack_qk(w):
        w2 = np.ascontiguousarray(w.transpose(1, 0, 2)).reshape(C, C)  # [c, h*64]
        w4 = w2.reshape(N_CC, 128, N_HP, 128).transpose(1, 2, 0, 3)   # [128c, hp, cc, 128]
        return np.ascontiguousarray(w4).reshape(128, -1).astype(proj_np)

    wq_h = pack_qk(Wq)
    wk_h = pack_qk(Wk)
    wv2 = np.ascontiguousarray(Wv.transpose(1, 0, 2)).reshape(C, C)    # [c, hd]
    wv_h = np.ascontiguousarray(
        wv2.reshape(N_CC, 128, C).transpose(1, 0, 2)).reshape(128, -1).astype(proj_np)
    wo_h = np.ascontiguousarray(
        Wo.reshape(N_CC, 128, C).transpose(1, 0, 2)).reshape(128, -1).astype(op_np)

    ii, jj = np.indices((128, 128))
    if cfg.get("mask_sbuf", False):
        mask_h = (jj >= ii).astype(np.float32)
    else:
        mask_h = np.where(jj >= ii, 0.0, NEG).astype(np.float32)
    ident_h = np.eye(128, dtype=np.float32)

    in_maps = []
    for core in range(N_CORES):
        xs = x[core * B_LOC:(core + 1) * B_LOC]              # [8, 256, 512]
        xT = np.ascontiguousarray(xs.transpose(0, 2, 1)).astype(proj_np)
        in_maps.append({
            "xT": xT, "wq": wq_h, "wk": wk_h, "wv": wv_h, "wo": wo_h,
            "mask": mask_h, "ident": ident_h,
        })
    return in_maps


DEFAULT_CFG = {"proj": "fp16", "scores": "fp16", "pv": "fp16", "outproj": "fp16"}

_NC_CACHE = {}


def run(x, Wq, Wk, Wv, Wo, cfg=None, trace=False):
    cfg = cfg or DEFAULT_CFG
    key = tuple(sorted(cfg.items()))
    if key not in _NC_CACHE:
        _NC_CACHE[key] = build_nc(cfg)
    nc = _NC_CACHE[key]
    in_maps = _prep_inputs(np.asarray(x), np.asarray(Wq), np.asarray(Wk),
                           np.asarray(Wv), np.asarray(Wo), cfg)
    res = run_bass_kernel_spmd(nc, in_maps, core_ids=list(range(N_CORES)),
                               trace=trace)
    y = np.concatenate([r["y"] for r in res.results], axis=0)
    return y, res


def kernel(x, Wq, Wk, Wv, Wo):
    y, _ = run(x, Wq, Wk, Wv, Wo)
    return y.astype(np.float32)


if __name__ == "__main__":
    import time
    t0 = time.time()
    nc = build_nc(DEFAULT_CFG)
    print(f"build+compile: {time.time()-t0:.1f}s")



# revision 17
# speedup vs baseline: 7.2719x; 3.0538x over previous
"""Trainium2 Bass kernel for causal multi-head attention.

Problem: x[64,256,512] f32, Wq/Wk/Wv[8,512,64], Wo[512,512]
  q,k,v = einsum('btc,hcd->bhtd'); scores = q k^T / sqrt(512) (causal);
  out = softmax(scores) v; y = concat-heads(out) @ Wo.

Strategy: data-parallel over batch across 8 NeuronCores (8 batches/core,
no collectives). Per core, for each batch b:
  - load host-pretransposed xT [512c, 256t]
  - QT/KT = W^T x^T via head-pair-packed matmuls -> [d, t] layout
    (two head-pairs share one PSUM bank -> one 512-wide copy out)
  - V' = x Wv -> [s, h, d] layout with a ones column appended per head
    (PV matmul then yields the softmax denominator for free)
  - per head: S^T[s,384] = [s0 x t(256) | s1 x t1(128)] in ONE psum tile;
    P = exp(scale*S) (single ACT op); multiplicative fp16 causal mask on
    the two diagonal blocks (gpsimd); fully-masked (t0,s1) never computed
  - PV: psum [t, 4head, 65] per (tb, head-quad); col 64 = denominator
  - normalize directly from PSUM: rec = 1/denoms (DVE), broadcast
    tensor_mul -> outn [t, tb, h, d] fp16
  - PE-transpose outn -> outT [hd, t]; y = outT^T Wo; DMA out.

Engine balance: exp is ACT-only; PSUM->SBUF copies are split between ACT
and DVE; causal mask multiplies run on gpsimd (SBUF-only engine).
Matmul operands are fp16 (1 cyc/row on PE vs 4 for fp32).
"""
import numpy as np

import concourse.bass as bass
import concourse.tile as tile
import concourse.mybir as mybir
from concourse import bacc
from concourse.bass_utils import run_bass_kernel_spmd

F32 = mybir.dt.float32
F16 = mybir.dt.float16
BF16 = mybir.dt.bfloat16

N_CORES = 8
B, T, C = 64, 256, 512
H, DK = 8, 64
B_LOC = B // N_CORES        # 8 batches per core
N_HP = H // 2               # head pairs (2x64 packed on partitions)
N_CC = C // 128             # contraction chunks
SCALE = 1.0 / np.sqrt(np.float32(C))

_DT = {"fp32": F32, "fp16": F16, "bf16": BF16}


def build_nc(cfg, repeat=0, tune=None):
    """repeat=0: straight-line kernel. repeat=R>0: wrap the whole per-batch
    pipeline in a hardware For_i loop executed R times (for timing)."""
    tu = {"xT": 2, "qk": 2, "vp": 2, "pp": 4, "op": 2, "rp": 4, "yp": 2,
          "ps_proj": 2, "ps_s": 3, "ps_pv": 3,
          # engine split for PSUM->SBUF copies: s=ACT, v=DVE, g=gpsimd
          "eng_qk": "svsv", "eng_vv": "sv", "eng_outT": "vvvv",
          "eng_yt": "sv", "eng_mask": "gggggggg",
          # mask_mode "mul": multiplicative fp16 mask after exp (eng_mask)
          # mask_mode "pe": additive -big mask folded into the scores
          #                 matmul accumulation on the tensor engine
          "mask_mode": "pe"}
    tu.update(tune or {})
    proj_dt = _DT[cfg["proj"]]     # x / Wq / Wk / Wv operand dtype
    sc_dt = _DT[cfg["scores"]]     # QT / KT operand dtype
    pv_dt = _DT[cfg["pv"]]         # P / V' operand dtype
    op_dt = _DT[cfg["outproj"]]    # outT / Wo operand dtype

    nc = bacc.Bacc("TRN2", target_bir_lowering=False, debug=False)

    def cp(engine, out, in_):
        if engine == "s":
            nc.scalar.copy(out, in_)
        elif engine == "v":
            nc.vector.tensor_copy(out, in_)
        else:
            nc.gpsimd.tensor_copy(out, in_)

    xT_d = nc.dram_tensor("xT", [B_LOC, C, T], proj_dt, kind="ExternalInput").ap()
    wq_d = nc.dram_tensor("wq", [128, N_HP * N_CC * 128], proj_dt, kind="ExternalInput").ap()
    wk_d = nc.dram_tensor("wk", [128, N_HP * N_CC * 128], proj_dt, kind="ExternalInput").ap()
    wv_d = nc.dram_tensor("wv", [128, N_CC * C], proj_dt, kind="ExternalInput").ap()
    wo_d = nc.dram_tensor("wo", [128, N_CC * C], op_dt, kind="ExternalInput").ap()
    mask_d = nc.dram_tensor("mask", [128, 384], pv_dt, kind="ExternalInput").ap()
    negm_d = nc.dram_tensor("negm", [128, 128], sc_dt, kind="ExternalInput").ap()
    ident_d = nc.dram_tensor("ident", [128, 128], op_dt, kind="ExternalInput").ap()
    identm_d = nc.dram_tensor("identm", [128, 128], sc_dt, kind="ExternalInput").ap()
    y_d = nc.dram_tensor("y", [B_LOC, T, C], F32, kind="ExternalOutput").ap()

    with tile.TileContext(nc) as tc:
        import contextlib
        ctx = contextlib.ExitStack()
        with ctx:
            const = ctx.enter_context(tc.tile_pool(name="const", bufs=1))
            xT_p = ctx.enter_context(tc.tile_pool(name="xT", bufs=tu["xT"]))
            qk_p = ctx.enter_context(tc.tile_pool(name="qk", bufs=tu["qk"]))
            vp_p = ctx.enter_context(tc.tile_pool(name="vp", bufs=tu["vp"]))
            p_p = ctx.enter_context(tc.tile_pool(name="pp", bufs=tu["pp"]))
            o_p = ctx.enter_context(tc.tile_pool(name="op", bufs=tu["op"]))
            r_p = ctx.enter_context(tc.tile_pool(name="rp", bufs=tu["rp"]))
            y_p = ctx.enter_context(tc.tile_pool(name="yp", bufs=tu["yp"]))
            ps_proj = ctx.enter_context(tc.tile_pool(name="ps_proj", bufs=tu["ps_proj"], space="PSUM"))
            ps_s = ctx.enter_context(tc.tile_pool(name="ps_s", bufs=tu["ps_s"], space="PSUM"))
            ps_pv = ctx.enter_context(tc.tile_pool(name="ps_pv", bufs=tu["ps_pv"], space="PSUM"))

            # persistent constants / weights
            mask = const.tile([128, 384], pv_dt)
            nc.sync.dma_start(mask[:], mask_d[:])
            negm = const.tile([128, 128], sc_dt)
            nc.sync.dma_start(negm[:], negm_d[:])
            ident = const.tile([128, 128], op_dt)
            nc.sync.dma_start(ident[:], ident_d[:])
            identm = const.tile([128, 128], sc_dt)
            nc.sync.dma_start(identm[:], identm_d[:])
            wq = const.tile([128, N_HP, N_CC, 128], proj_dt)
            nc.sync.dma_start(wq[:], wq_d.rearrange("p (a b c) -> p a b c", a=N_HP, b=N_CC))
            wk = const.tile([128, N_HP, N_CC, 128], proj_dt)
            nc.sync.dma_start(wk[:], wk_d.rearrange("p (a b c) -> p a b c", a=N_HP, b=N_CC))
            wv = const.tile([128, N_CC, C], proj_dt)
            nc.sync.dma_start(wv[:], wv_d.rearrange("p (a b) -> p a b", a=N_CC))
            wo = const.tile([128, N_CC, C], op_dt)
            nc.sync.dma_start(wo[:], wo_d.rearrange("p (a b) -> p a b", a=N_CC))

            def emit_load(st):
                # ---- load xT for this batch: [128, cc, 256] ----
                st["xT"] = xT = xT_p.tile([128, N_CC, T], proj_dt, tag="xT",
                                          name="xT_t")
                nc.sync.dma_start(
                    xT[:], xT_d[st["b"]].rearrange("(a p) t -> p a t", p=128))

            def emit_proj_group(st, gi):
                # ---- projections, one PSUM-bank group at a time ----
                # gi 0,1: Q head-pair-pairs; 2,3: K; 4,5: V s-chunks.
                # Two head-pairs share one PSUM bank -> one 512-wide copy.
                xT = st["xT"]
                if gi < 4:
                    w = (wq, wk)[gi // 2]
                    if gi == 0:
                        st["qt"] = qk_p.tile([128, N_HP, T], sc_dt, tag="qt",
                                             name="qt_t")
                    if gi == 2:
                        st["kt"] = qk_p.tile([128, N_HP, T], sc_dt, tag="kt",
                                             name="kt_t")
                    dst = st["qt"] if gi < 2 else st["kt"]
                    hpp = gi % 2
                    ps = ps_proj.tile([128, 512], F32, tag="ps_proj")
                    for hp2 in range(2):
                        hp = 2 * hpp + hp2
                        for cc in range(N_CC):
                            nc.tensor.matmul(
                                ps[:, hp2 * T:(hp2 + 1) * T],
                                w[:, hp, cc, :], xT[:, cc, :],
                                start=(cc == 0), stop=(cc == N_CC - 1))
                    cp(tu["eng_qk"][gi],
                       dst[:, 2 * hpp:2 * hpp + 2, :],
                       ps[:].rearrange("p (a t) -> p a t", a=2))
                else:
                    # V projection -> V' [128s, schunk, h, 65]
                    sc = gi - 4
                    if sc == 0:
                        st["vv"] = vp_p.tile([128, 2, H, 65], pv_dt, tag="vv",
                                             name="vv_t")
                    vv = st["vv"]
                    ps = ps_proj.tile([128, 512], F32, tag="ps_proj")
                    for cc in range(N_CC):
                        nc.tensor.matmul(
                            ps[:], xT[:, cc, bass.ts(sc, 128)], wv[:, cc, :],
                            start=(cc == 0), stop=(cc == N_CC - 1))
                    cp(tu["eng_vv"][sc],
                       vv[:, sc, :, 0:DK],
                       ps[:].rearrange("p (h d) -> p h d", h=H))
                    if sc == 1:
                        nc.vector.memset(vv[:, :, :, DK:65], 1.0)

            def emit_scores(st, h):
                qt, kt = st["qt"], st["kt"]
                hp, lo = h // 2, (h % 2) * DK
                qs = qt[lo:lo + DK, hp, :]
                ks = kt[lo:lo + DK, hp, :]
                # S^T in one tile: [s0 x t(0:256) | s1 x t1(256:384)]
                ss = ps_s.tile([128, 384], F32, tag="s", name=f"s_{h}")
                if tu["mask_mode"] == "pe":
                    # additive -big causal mask folded into the PSUM
                    # accumulation of the two diagonal blocks
                    nc.tensor.matmul(ss[:, 0:128], ks[:, 0:128],
                                     qs[:, 0:128], start=True, stop=False)
                    nc.tensor.matmul(ss[:, 0:128], identm[:], negm[:],
                                     start=False, stop=True)
                    nc.tensor.matmul(ss[:, 128:256], ks[:, 0:128],
                                     qs[:, 128:256], start=True, stop=True)
                    nc.tensor.matmul(ss[:, 256:384], ks[:, 128:256],
                                     qs[:, 128:256], start=True, stop=False)
                    nc.tensor.matmul(ss[:, 256:384], identm[:], negm[:],
                                     start=False, stop=True)
                else:
                    nc.tensor.matmul(ss[:, 0:T], ks[:, 0:128], qs[:],
                                     start=True, stop=True)
                    nc.tensor.matmul(ss[:, T:384], ks[:, 128:256],
                                     qs[:, 128:256], start=True, stop=True)
                # P = exp(scale * S)
                p = p_p.tile([128, 384], pv_dt, tag="p", name=f"p_{h}")
                nc.scalar.activation(p[:], ss[:],
                                     mybir.ActivationFunctionType.Exp,
                                     bias=0.0, scale=float(SCALE))
                if tu["mask_mode"] == "mul":
                    me = tu["eng_mask"][h]
                    mop = (nc.gpsimd if me == "g" else nc.vector).tensor_mul
                    mop(p[:], p[:], mask[:])
                st[f"p{h}"] = p

            def emit_pv(st, h):
                vv, p = st["vv"], st.pop(f"p{h}")
                outn, pvt = st["outn"], st["pvt"]
                # PV into per-(tb, head-quad) psum [t, 4, 65]
                tb_jobs = ((0, ((p[:, 0:128], 0, True, True),)),
                           (1, ((p[:, 128:256], 0, True, False),
                                (p[:, 256:384], 1, False, True))))
                for tb, jobs in tb_jobs:
                    key, q4 = (tb, h // 4), h % 4
                    if q4 == 0:
                        pvt[key] = ps_pv.tile([128, 4, 65], F32, tag="pv",
                                              name=f"pv_{tb}_{h // 4}")
                    for (pslice, sc, st_, sp) in jobs:
                        nc.tensor.matmul(pvt[key][:, q4, :], pslice,
                                         vv[:, sc, h, :],
                                         start=st_, stop=sp)
                    if q4 == 3:
                        # normalize straight out of PSUM
                        tile_ = pvt.pop(key)
                        rec = r_p.tile([128, 4, 1], F32, tag="rec")
                        nc.vector.reciprocal(rec[:, :, 0], tile_[:, :, DK])
                        nc.vector.tensor_mul(
                            outn[:, tb, 4 * (h // 4):4 * (h // 4) + 4, :],
                            tile_[:, :, 0:DK],
                            rec[:].to_broadcast([128, 4, DK]))

            def emit_transp(st, ccp):
                # ---- transpose outn [t,(h d)] -> outT [hd, tb*128] ----
                outn, outT = st["outn"], st["outT"]
                for tb in range(2):
                    tp = ps_s.tile([128, 256], op_dt, tag="s",
                                   name=f"tp_{tb}_{ccp}")
                    for c2 in range(2):
                        cc = 2 * ccp + c2
                        nc.tensor.transpose(
                            tp[:, bass.ts(c2, 128)],
                            outn[:, tb, 2 * cc:2 * cc + 2, :], ident[:])
                    cp(tu["eng_outT"][2 * tb + ccp],
                       outT[:, 2 * ccp:2 * ccp + 2, bass.ts(tb, 128)],
                       tp[:].rearrange("p (a t) -> p a t", a=2))

            def emit_heads(st, nxt):
                # head loop for batch b, with the next batch's projection
                # groups interleaved into the PE idle slots (the head loop
                # is ACT-paced: exp > per-head PE work).
                st["outn"] = o_p.tile([128, 2, H, DK], op_dt, tag="outn",
                                      name="outn_t")
                st["outT"] = o_p.tile([128, N_CC, T], op_dt, tag="outT",
                                      name="outT_t")
                st["pvt"] = {}
                emit_scores(st, 0)
                for h in range(H):
                    if h + 1 < H:
                        emit_scores(st, h + 1)
                    emit_pv(st, h)
                    if nxt is not None and h < 5:
                        emit_proj_group(nxt, h)
                    if h == 5:
                        emit_transp(st, 0)

            def emit_tail(st, nxt):
                # V s-chunk-1 projection of the next batch covers the
                # normalize latency of head-quad 1 before its transposes
                if nxt is not None:
                    emit_proj_group(nxt, 5)
                emit_transp(st, 1)
                # ---- output projection ----
                outT, b = st["outT"], st["b"]
                for tb in range(2):
                    ps = ps_proj.tile([128, 512], F32, tag="ps_proj")
                    for cc in range(N_CC):
                        nc.tensor.matmul(
                            ps[:], outT[:, cc, bass.ts(tb, 128)], wo[:, cc, :],
                            start=(cc == 0), stop=(cc == N_CC - 1))
                    yt = y_p.tile([128, C], F32, tag="yt")
                    cp(tu["eng_yt"][tb], yt[:], ps[:])
                    nc.sync.dma_start(y_d[b, bass.ts(tb, 128), :], yt[:])

            def batch_pipeline():
                # software pipeline across batches: batch b+1's projections
                # are interleaved into batch b's head loop and tail so PE
                # never waits on the exp/normalize chains.
                sts = [{"b": b} for b in range(B_LOC)]
                emit_load(sts[0])
                for gi in range(6):
                    emit_proj_group(sts[0], gi)
                for b in range(B_LOC):
                    nxt = sts[b + 1] if b + 1 < B_LOC else None
                    if nxt is not None:
                        emit_load(nxt)
                    emit_heads(sts[b], nxt)
                    emit_tail(sts[b], nxt)

            if repeat:
                with tc.For_i(0, repeat, 1):
                    batch_pipeline()
            else:
                batch_pipeline()

    nc.compile()
    return nc


def _prep_inputs(x, Wq, Wk, Wv, Wo, cfg):
    """Host-side reshapes/casts. Returns per-core input maps."""
    def np_dt(name):
        if name == "fp16":
            return np.float16
        if name == "bf16":
            import ml_dtypes
            return ml_dtypes.bfloat16
        return np.float32

    proj_np = np_dt(cfg["proj"])
    pv_np = np_dt(cfg["pv"])
    op_np = np_dt(cfg["outproj"])

    # weights: head-pair stationary blocks [hp, cc, 128c, 128d2] -> [128, hp*cc*128]
    def pack_qk(w):
        w2 = np.ascontiguousarray(w.transpose(1, 0, 2)).reshape(C, C)  # [c, h*64]
        w4 = w2.reshape(N_CC, 128, N_HP, 128).transpose(1, 2, 0, 3)   # [128c, hp, cc, 128]
        return np.ascontiguousarray(w4).reshape(128, -1).astype(proj_np)

    wq_h = pack_qk(Wq)
    wk_h = pack_qk(Wk)
    wv2 = np.ascontiguousarray(Wv.transpose(1, 0, 2)).reshape(C, C)    # [c, hd]
    wv_h = np.ascontiguousarray(
        wv2.reshape(N_CC, 128, C).transpose(1, 0, 2)).reshape(128, -1).astype(proj_np)
    wo_h = np.ascontiguousarray(
        Wo.reshape(N_CC, 128, C).transpose(1, 0, 2)).reshape(128, -1).astype(op_np)

    ii, jj = np.indices((128, 128))
    tri = (jj >= ii).astype(np.float32)     # [s, t]: keep t >= s
    mask_h = np.concatenate(
        [tri, np.ones((128, 128), np.float32), tri], axis=1).astype(pv_np)
    sc_np = np_dt(cfg["scores"])
    negm_h = np.where(jj >= ii, 0.0, -60000.0).astype(sc_np)
    ident_h = np.eye(128, dtype=np.float32).astype(op_np)
    identm_h = np.eye(128, dtype=np.float32).astype(sc_np)

    in_maps = []
    for core in range(N_CORES):
        xs = x[core * B_LOC:(core + 1) * B_LOC]              # [8, 256, 512]
        xT = np.ascontiguousarray(xs.transpose(0, 2, 1)).astype(proj_np)
        in_maps.append({
            "xT": xT, "wq": wq_h, "wk": wk_h, "wv": wv_h, "wo": wo_h,
            "mask": mask_h, "negm": negm_h, "ident": ident_h,
            "identm": identm_h,
        })
    return in_maps


DEFAULT_CFG = {"proj": "fp16", "scores": "fp16", "pv": "fp16", "outproj": "fp16"}

_NC_CACHE = {}


def run(x, Wq, Wk, Wv, Wo, cfg=None, trace=False):
    cfg = cfg or DEFAULT_CFG
    key = tuple(sorted(cfg.items()))
    if key not in _NC_CACHE:
        _NC_CACHE[key] = build_nc(cfg)
    nc = _NC_CACHE[key]
    in_maps = _prep_inputs(np.asarray(x), np.asarray(Wq), np.asarray(Wk),
                           np.asarray(Wv), np.asarray(Wo), cfg)
    res = run_bass_kernel_spmd(nc, in_maps, core_ids=list(range(N_CORES)),
                               trace=trace)
    y = np.concatenate([r["y"] for r in res.results], axis=0)
    return y, res


def kernel(x, Wq, Wk, Wv, Wo):
    y, _ = run(x, Wq, Wk, Wv, Wo)
    return y.astype(np.float32)


if __name__ == "__main__":
    import time
    t0 = time.time()
    nc = build_nc(DEFAULT_CFG)
    print(f"build+compile: {time.time()-t0:.1f}s")
